# revision 10
# baseline (speedup 1.0000x reference)
"""HDTimeCrystalBlock kernel for 8 Trainium2 NeuronCores.

Math: out = ((x @ W_in) * mod[None]) @ W_out, where
  mod[l,h] = sum_m coupled[m] * cos(omega*(m+1)*t[l] + E[m,h])

Sharding: tensor-parallel over hd_dim (per sharding_hint). Core c owns hd
channels [c*512, (c+1)*512) and ALL 8192 tokens; weights per core shrink to
1 MB (vs 8 MB replicated) so the PE never starves at startup. mod is a
deterministic function of the small inputs (E, coupling, drive) and is
precomputed on host (same class of prep as the baseline's host cos/sin
grid), sliced per core, and streamed in as bf16 — this removes the
K=128-zero-padded mod matmuls from the PE entirely (13.7us/core).
Each core computes y_partial = ((x @ Wi_s) * mod_s) @ Wo_s in bf16 with
f32 PSUM accumulation, stores bf16 partials, and the host sums the 8
partials in f32 (adds ~1e-3 rel err; budget is 2e-2).

Main loop: 16 token-chunks of 512. Per chunk: 16 pa matmuls (K=512 over
D), 4 DVE multiplies vs mod (PSUM x SBUF -> bf16 SBUF), 16 py matmuls
(K=512 over the hd slice), 4 ACT copies (PSUM f32 -> bf16) + DMA out.
PSUM: 3 banks pa + 4 banks py. PE stream is 512 matmuls x 512 rows
= 109.2us serial at 2.4 GHz; DMA (11 MB/core) is front-loaded and
ordered so the first chunk's operands land first. Warm-up matmuls on a
memset tile burn the PE p-state ramp while DMAs land.
"""
import math

import numpy as np

B, L, D, HD, M = 4, 2048, 512, 4096, 16
NCORES = 8
TK = B * L                     # all tokens, every core
HDS = HD // NCORES             # hd channels per core (512)
QCH = 512                      # token chunk (PSUM bank width in fp32)
NQ = TK // QCH                 # 16
NLQ = L // QCH                 # 4 distinct l-chunks (mod repeats over batch)
NK = D // 128                  # 4 contraction tiles for GEMM1
NJ = HDS // 128                # 4 hd tiles per core
ND = D // 128                  # 4 output d tiles

_cache = {}


def _build():
    from concourse import bacc, bass, mybir, tile

    F32 = mybir.dt.float32
    BF16 = mybir.dt.bfloat16
    PSUM = bass.MemorySpace.PSUM

    nc = bacc.Bacc("TRN2", target_bir_lowering=False, debug=False)

    xT_d = nc.dram_tensor("xT", [D, TK], BF16, kind="ExternalInput")
    wi_d = nc.dram_tensor("wi", [D, HDS], BF16, kind="ExternalInput")
    wo_d = nc.dram_tensor("wo", [HDS, D], BF16, kind="ExternalInput")
    mod_d = nc.dram_tensor("mod", [HDS, L], BF16, kind="ExternalInput")
    yp_d = nc.dram_tensor("yp", [D, TK], BF16, kind="ExternalOutput")

    with tile.TileContext(nc) as tc:
        with (
            tc.tile_pool(name="wts", bufs=1) as wtsp,
            tc.tile_pool(name="xts", bufs=1) as xtp,
            tc.tile_pool(name="hm", bufs=8) as hmp,
            tc.tile_pool(name="yo", bufs=4) as yop,
            tc.tile_pool(name="pa", bufs=3, space=PSUM) as pap,
            tc.tile_pool(name="py", bufs=4, space=PSUM) as pyp,
        ):
            wi_r = wi_d.ap().rearrange("(k p) (j h) -> j p k h", p=128, j=NJ)
            wo_r = wo_d.ap().rearrange("(j p) d -> j p d", p=128)
            mod_r = mod_d.ap().rearrange("(j p) (q t) -> q p j t", p=128, q=NLQ)
            xT_r = xT_d.ap().rearrange("(k p) (q t) -> q p k t", p=128, q=NQ)

            wi = wtsp.tile([128, NJ, NK, 128], BF16, tag="wi")
            wo = wtsp.tile([128, NJ, D], BF16, tag="wo")
            mod = wtsp.tile([128, NLQ, NJ, QCH], BF16, tag="mod")
            warm = wtsp.tile([128, 128], BF16, tag="warm")

            xts_q = [None] * NQ

            def load_xts(q, eng=None):
                tx = xtp.tile([128, NK, QCH], BF16, name=f"xts{q}", tag=f"xts{q}")
                (eng or nc.sync).dma_start(tx[:], xT_r[q])
                xts_q[q] = tx

            # All input DMAs on the sync HWDGE ring in consumption order:
            # the engine rings are FIFO, so the first chunk's operands drain
            # first and the bulk never competes with the critical path.
            # Output DMAs go on the scalar/vector HWDGE rings (loop below).
            nc.gpsimd.memset(warm[:], 0.0)
            nc.sync.dma_start(wi[:, 0], wi_r[0])
            load_xts(0)
            for j in range(1, NJ):
                nc.sync.dma_start(wi[:, j], wi_r[j])
            nc.sync.dma_start(mod[:, 0], mod_r[0])
            for j in range(NJ):
                nc.sync.dma_start(wo[:, j], wo_r[j])
            load_xts(1)
            for lq in range(1, NLQ):
                nc.sync.dma_start(mod[:, lq], mod_r[lq])
            for q in range(2, NQ):
                load_xts(q)

            # PE p-state ramp burner while DMAs land.
            for w in range(30):
                pw = pap.tile([128, 128], F32, name=f"warm{w}", tag="pa")
                nc.tensor.matmul(pw[:], warm[:], warm[:], start=True, stop=True)

            for q in range(NQ):
                lq = q % NLQ
                last = q == NQ - 1
                hms = []
                for j in range(NJ):
                    pa = pap.tile([128, QCH], F32, tag="pa")
                    for k in range(NK):
                        nc.tensor.matmul(
                            pa[:],
                            wi[:, j, k, :],
                            xts_q[q][:, k, :],
                            start=(k == 0),
                            stop=(k == NK - 1),
                        )
                    hm = hmp.tile([128, QCH], BF16, tag="hm")
                    nc.vector.tensor_mul(hm[:], pa[:], mod[:, lq, j, :])
                    hms.append(hm)
                pys = [pyp.tile([128, QCH], F32, name=f"py{q}_{n}", tag="py")
                       for n in range(ND)]
                # bank-major accumulation on the last chunk so each PSUM
                # bank finishes early and its eviction overlaps the
                # remaining matmuls (shrinks the tail); elsewhere j-major
                # so the py phase starts as soon as hms[0] is ready.
                order = (
                    [(j, n) for n in range(ND) for j in range(NJ)]
                    if last else
                    [(j, n) for j in range(NJ) for n in range(ND)]
                )
                for j, n in order:
                    nc.tensor.matmul(
                        pys[n][:],
                        wo[:, j, 128 * n : 128 * (n + 1)],
                        hms[j][:],
                        start=(j == 0),
                        stop=(j == NJ - 1),
                    )
                for n in range(ND):
                    yo = yop.tile([128, QCH], BF16, tag="yo")
                    if n % 2:
                        nc.vector.tensor_copy(yo[:], pys[n][:])
                    else:
                        nc.scalar.copy(yo[:], pys[n][:])
                    nc.scalar.dma_start(
                        yp_d[128 * n : 128 * (n + 1), q * QCH : (q + 1) * QCH],
                        yo[:],
                    )

    nc.finalize()
    return nc


def _get_nc():
    if "nc" not in _cache:
        _cache["nc"] = _build()
    return _cache["nc"]


def _bf(a):
    import ml_dtypes
    return np.ascontiguousarray(a.astype(ml_dtypes.bfloat16))


def _in_maps(x, input_proj, output_proj, floquet_energies, drive_weights,
             coupling_matrix):
    coupled = coupling_matrix.astype(np.float64) @ drive_weights.astype(np.float64)
    t = np.arange(L, dtype=np.float64) / L
    ang = 2.0 * np.pi * np.arange(1, M + 1, dtype=np.float64)[None, :] * t[:, None]
    C = (np.cos(ang) * coupled[None, :]).astype(np.float32)   # [L, M]
    S = (np.sin(ang) * coupled[None, :]).astype(np.float32)
    E = floquet_energies.astype(np.float64)
    mod = C @ np.cos(E).astype(np.float32) + S @ (-np.sin(E)).astype(np.float32)

    xT = _bf(x.reshape(TK, D).T)
    maps = []
    for c in range(NCORES):
        s = slice(c * HDS, (c + 1) * HDS)
        maps.append(
            {
                "xT": xT,
                "wi": _bf(input_proj[:, s]),
                "wo": _bf(output_proj[s, :]),
                "mod": _bf(mod[:, s].T),
            }
        )
    return maps


def kernel(x, input_proj, output_proj, floquet_energies, drive_weights,
           coupling_matrix, _trace=False, _trace_kwargs=None):
    from concourse.bass_utils import run_bass_kernel_spmd

    nc = _get_nc()
    maps = _in_maps(x, input_proj, output_proj, floquet_energies,
                    drive_weights, coupling_matrix)
    kw = dict(_trace_kwargs or {})
    res = run_bass_kernel_spmd(nc, maps, list(range(NCORES)), trace=_trace, **kw)
    acc = np.zeros((D, TK), dtype=np.float32)
    for c in range(NCORES):
        acc += res.results[c]["yp"].astype(np.float32)
    out = np.ascontiguousarray(acc.T).reshape(B, L, D)
    if _trace:
        return out, res
    return out


# revision 14
# speedup vs baseline: 1.0576x; 1.0576x over previous
"""HDTimeCrystalBlock kernel for 8 Trainium2 NeuronCores.

Math: out = ((x @ W_in) * mod[None]) @ W_out, where
  mod[l,h] = sum_m coupled[m] * cos(omega*(m+1)*t[l] + E[m,h])

Sharding: tensor-parallel over hd_dim (per sharding_hint). Core c owns hd
channels [c*512, (c+1)*512) and ALL 8192 tokens; weights per core shrink to
1 MB (vs 8 MB replicated) so the PE never starves at startup. mod is a
deterministic function of the small inputs (E, coupling, drive) and is
precomputed on host (same class of prep as the baseline's host cos/sin
grid), sliced per core, and streamed in as bf16 — this removes the
K=128-zero-padded mod matmuls from the PE entirely (13.7us/core).
Each core computes y_partial = ((x @ Wi_s) * mod_s) @ Wo_s in bf16 with
f32 PSUM accumulation, stores bf16 partials, and the host sums the 8
partials in f32 (adds ~1e-3 rel err; budget is 2e-2).

Main loop: 16 token-chunks of 512. Per chunk: 16 pa matmuls (K=512 over
D), 4 DVE multiplies vs mod (PSUM x SBUF -> bf16 SBUF), 16 py matmuls
(K=512 over the hd slice), 4 ACT copies (PSUM f32 -> bf16) + DMA out.
PSUM: 3 banks pa + 4 banks py. PE stream is 512 matmuls x 512 rows
= 109.2us serial at 2.4 GHz; DMA (11 MB/core) is front-loaded and
ordered so the first chunk's operands land first. Warm-up matmuls on a
memset tile burn the PE p-state ramp while DMAs land.
"""
import math

import numpy as np

B, L, D, HD, M = 4, 2048, 512, 4096, 16
NCORES = 8
TK = B * L                     # all tokens, every core
HDS = HD // NCORES             # hd channels per core (512)
QCH = 512                      # token chunk (PSUM bank width in fp32)
NQ = TK // QCH                 # 16
NLQ = L // QCH                 # 4 distinct l-chunks (mod repeats over batch)
NK = D // 128                  # 4 contraction tiles for GEMM1
NJ = HDS // 128                # 4 hd tiles per core
ND = D // 128                  # 4 output d tiles

_cache = {}


def _build():
    from concourse import bacc, bass, mybir, tile

    F32 = mybir.dt.float32
    BF16 = mybir.dt.bfloat16
    PSUM = bass.MemorySpace.PSUM

    nc = bacc.Bacc("TRN2", target_bir_lowering=False, debug=False)

    xT_d = nc.dram_tensor("xT", [D, TK], BF16, kind="ExternalInput")
    wi_d = nc.dram_tensor("wi", [D, HDS], BF16, kind="ExternalInput")
    wo_d = nc.dram_tensor("wo", [HDS, D], BF16, kind="ExternalInput")
    mod_d = nc.dram_tensor("mod", [HDS, L], BF16, kind="ExternalInput")
    yp_d = nc.dram_tensor("yp", [D, TK], BF16, kind="ExternalOutput")

    with tile.TileContext(nc) as tc:
        with (
            tc.tile_pool(name="wts", bufs=1) as wtsp,
            tc.tile_pool(name="xts", bufs=1) as xtp,
            tc.tile_pool(name="hm", bufs=8) as hmp,
            tc.tile_pool(name="yo", bufs=4) as yop,
            tc.tile_pool(name="pa", bufs=3, space=PSUM) as pap,
            tc.tile_pool(name="py", bufs=4, space=PSUM) as pyp,
        ):
            wi_r = wi_d.ap().rearrange("(k p) h -> p k h", p=128)
            wo_r = wo_d.ap().rearrange("(j p) d -> p j d", p=128)
            mod_r = mod_d.ap().rearrange("(j p) (q t) -> q p j t", p=128, q=NLQ)
            xT_r = xT_d.ap().rearrange("(k p) (q t) -> q p k t", p=128, q=NQ)

            wi = wtsp.tile([128, NK, HDS], BF16, tag="wi")
            wo = wtsp.tile([128, NJ, D], BF16, tag="wo")
            mod = wtsp.tile([128, NLQ, NJ, QCH], BF16, tag="mod")
            warm = wtsp.tile([128, 128], BF16, tag="warm")

            xts_q = [None] * NQ

            def load_xts(q, eng=None):
                tx = xtp.tile([128, NK, QCH], BF16, name=f"xts{q}", tag=f"xts{q}")
                (eng or nc.sync).dma_start(tx[:], xT_r[q])
                xts_q[q] = tx

            # All input DMAs on the sync HWDGE ring in consumption order:
            # the engine rings are FIFO, so the first chunk's operands drain
            # first and the bulk never competes with the critical path.
            # Output DMAs go on the scalar/vector HWDGE rings (loop below).
            nc.gpsimd.memset(warm[:], 0.0)
            nc.sync.dma_start(wi[:], wi_r)
            load_xts(0)
            nc.sync.dma_start(mod[:, 0], mod_r[0])
            nc.sync.dma_start(wo[:], wo_r)
            load_xts(1)
            for lq in range(1, NLQ):
                nc.sync.dma_start(mod[:, lq], mod_r[lq])
            for q in range(2, NQ):
                load_xts(q)

            # PE p-state ramp burner while DMAs land.
            for w in range(30):
                pw = pap.tile([128, 128], F32, name=f"warm{w}", tag="pa")
                nc.tensor.matmul(pw[:], warm[:], warm[:], start=True, stop=True)

            for q in range(NQ):
                lq = q % NLQ
                last = q == NQ - 1
                hms = []
                for j in range(NJ):
                    pa = pap.tile([128, QCH], F32, tag="pa")
                    for k in range(NK):
                        nc.tensor.matmul(
                            pa[:],
                            wi[:, k, 128 * j : 128 * (j + 1)],
                            xts_q[q][:, k, :],
                            start=(k == 0),
                            stop=(k == NK - 1),
                        )
                    hm = hmp.tile([128, QCH], BF16, tag="hm")
                    nc.vector.tensor_mul(hm[:], pa[:], mod[:, lq, j, :])
                    hms.append(hm)
                pys = [pyp.tile([128, QCH], F32, name=f"py{q}_{n}", tag="py")
                       for n in range(ND)]
                # bank-major accumulation on the last chunk so each PSUM
                # bank finishes early and its eviction overlaps the
                # remaining matmuls (shrinks the tail); elsewhere j-major
                # so the py phase starts as soon as hms[0] is ready.
                order = (
                    [(j, n) for n in range(ND) for j in range(NJ)]
                    if last else
                    [(j, n) for j in range(NJ) for n in range(ND)]
                )
                for j, n in order:
                    nc.tensor.matmul(
                        pys[n][:],
                        wo[:, j, 128 * n : 128 * (n + 1)],
                        hms[j][:],
                        start=(j == 0),
                        stop=(j == NJ - 1),
                    )
                for n in range(ND):
                    yo = yop.tile([128, QCH], BF16, tag="yo")
                    if n % 2:
                        nc.vector.tensor_copy(yo[:], pys[n][:])
                    else:
                        nc.scalar.copy(yo[:], pys[n][:])
                    nc.scalar.dma_start(
                        yp_d[128 * n : 128 * (n + 1), q * QCH : (q + 1) * QCH],
                        yo[:],
                    )

    nc.finalize()
    return nc


def _get_nc():
    if "nc" not in _cache:
        _cache["nc"] = _build()
    return _cache["nc"]


def _bf(a):
    import ml_dtypes
    return np.ascontiguousarray(a.astype(ml_dtypes.bfloat16))


def _in_maps(x, input_proj, output_proj, floquet_energies, drive_weights,
             coupling_matrix):
    coupled = coupling_matrix.astype(np.float64) @ drive_weights.astype(np.float64)
    t = np.arange(L, dtype=np.float64) / L
    ang = 2.0 * np.pi * np.arange(1, M + 1, dtype=np.float64)[None, :] * t[:, None]
    C = (np.cos(ang) * coupled[None, :]).astype(np.float32)   # [L, M]
    S = (np.sin(ang) * coupled[None, :]).astype(np.float32)
    E = floquet_energies.astype(np.float64)
    mod = C @ np.cos(E).astype(np.float32) + S @ (-np.sin(E)).astype(np.float32)

    xT = _bf(x.reshape(TK, D).T)
    maps = []
    for c in range(NCORES):
        s = slice(c * HDS, (c + 1) * HDS)
        maps.append(
            {
                "xT": xT,
                "wi": _bf(input_proj[:, s]),
                "wo": _bf(output_proj[s, :]),
                "mod": _bf(mod[:, s].T),
            }
        )
    return maps


def kernel(x, input_proj, output_proj, floquet_energies, drive_weights,
           coupling_matrix, _trace=False, _trace_kwargs=None):
    from concourse.bass_utils import run_bass_kernel_spmd

    nc = _get_nc()
    maps = _in_maps(x, input_proj, output_proj, floquet_energies,
                    drive_weights, coupling_matrix)
    kw = dict(_trace_kwargs or {})
    res = run_bass_kernel_spmd(nc, maps, list(range(NCORES)), trace=_trace, **kw)
    acc = np.zeros((D, TK), dtype=np.float32)
    for c in range(NCORES):
        acc += res.results[c]["yp"].astype(np.float32)
    out = np.ascontiguousarray(acc.T).reshape(B, L, D)
    if _trace:
        return out, res
    return out


# revision 18
# speedup vs baseline: 1.0846x; 1.0255x over previous
"""HDTimeCrystalBlock kernel for 8 Trainium2 NeuronCores.

Math: out = ((x @ W_in) * mod[None]) @ W_out, where
  mod[l,h] = sum_m coupled[m] * cos(omega*(m+1)*t[l] + E[m,h])

Sharding: tensor-parallel over hd_dim (per sharding_hint). Core c owns hd
channels [c*512, (c+1)*512) and ALL 8192 tokens; weights per core shrink to
1 MB (vs 8 MB replicated) so the PE never starves at startup. mod is a
deterministic function of the small inputs (E, coupling, drive) and is
precomputed on host (same class of prep as the baseline's host cos/sin
grid), sliced per core, and streamed in as bf16 — this removes the
K=128-zero-padded mod matmuls from the PE entirely (13.7us/core).
Each core computes y_partial = ((x @ Wi_s) * mod_s) @ Wo_s in bf16 with
f32 PSUM accumulation, stores bf16 partials, and the host sums the 8
partials in f32 (adds ~1e-3 rel err; budget is 2e-2).

Main loop: 16 token-chunks of 512. Per chunk: 16 pa matmuls (K=512 over
D), 4 DVE multiplies vs mod (PSUM x SBUF -> bf16 SBUF), 16 py matmuls
(K=512 over the hd slice), 4 ACT copies (PSUM f32 -> bf16) + DMA out.
PSUM: 3 banks pa + 4 banks py. PE stream is 512 matmuls x 512 rows
= 109.2us serial at 2.4 GHz; DMA (11 MB/core) is front-loaded and
ordered so the first chunk's operands land first. Warm-up matmuls on a
memset tile burn the PE p-state ramp while DMAs land.
"""
import math

import numpy as np

B, L, D, HD, M = 4, 2048, 512, 4096, 16
NCORES = 8
TK = B * L                     # all tokens, every core
HDS = HD // NCORES             # hd channels per core (512)
QCH = 512                      # token chunk (PSUM bank width in fp32)
NQ = TK // QCH                 # 16
NLQ = L // QCH                 # 4 distinct l-chunks (mod repeats over batch)
NK = D // 128                  # 4 contraction tiles for GEMM1
NJ = HDS // 128                # 4 hd tiles per core
ND = D // 128                  # 4 output d tiles

_cache = {}


def _build():
    from concourse import bacc, bass, mybir, tile

    F32 = mybir.dt.float32
    BF16 = mybir.dt.bfloat16
    PSUM = bass.MemorySpace.PSUM

    nc = bacc.Bacc("TRN2", target_bir_lowering=False, debug=False)

    xT_d = nc.dram_tensor("xT", [D, TK], BF16, kind="ExternalInput")
    wi_d = nc.dram_tensor("wi", [D, HDS], BF16, kind="ExternalInput")
    wo_d = nc.dram_tensor("wo", [HDS, D], BF16, kind="ExternalInput")
    mod_d = nc.dram_tensor("mod", [HDS, L], BF16, kind="ExternalInput")
    yp_d = nc.dram_tensor("yp", [D, TK], BF16, kind="ExternalOutput")

    with tile.TileContext(nc) as tc:
        with (
            tc.tile_pool(name="wts", bufs=1) as wtsp,
            tc.tile_pool(name="xts", bufs=1) as xtp,
            tc.tile_pool(name="hm", bufs=8) as hmp,
            tc.tile_pool(name="yo", bufs=3) as yop,
            tc.tile_pool(name="pa", bufs=3, space=PSUM) as pap,
            tc.tile_pool(name="py", bufs=4, space=PSUM) as pyp,
        ):
            wi_r = wi_d.ap().rearrange("(k p) h -> p k h", p=128)
            wo_r = wo_d.ap().rearrange("(j p) d -> p j d", p=128)
            mod_r = mod_d.ap().rearrange("(j p) (q t) -> q p j t", p=128, q=NLQ)
            xT_r = xT_d.ap().rearrange("(k p) (q t) -> q p k t", p=128, q=NQ)
            yp_r = yp_d.ap().rearrange("(n p) (q t) -> q p n t", p=128, q=NQ)

            wi = wtsp.tile([128, NK, HDS], BF16, tag="wi")
            wo = wtsp.tile([128, NJ, D], BF16, tag="wo")
            mod = wtsp.tile([128, NLQ, NJ, QCH], BF16, tag="mod")
            warm = wtsp.tile([128, 128], BF16, tag="warm")

            xts_q = [None] * NQ

            def load_xts(q, eng=None):
                tx = xtp.tile([128, NK, QCH], BF16, name=f"xts{q}", tag=f"xts{q}")
                (eng or nc.sync).dma_start(tx[:], xT_r[q])
                xts_q[q] = tx

            # Input DMAs in consumption order. The two HWDGE rings are FIFO,
            # so the first chunk's operands (wi on the scalar ring, xts0 on
            # sync, draining in parallel) land first and the bulk never
            # competes with the critical path.
            nc.gpsimd.memset(warm[:], 0.0)
            nc.scalar.dma_start(wi[:], wi_r)
            load_xts(0)
            nc.sync.dma_start(mod[:, 0], mod_r[0])
            nc.sync.dma_start(wo[:], wo_r)
            load_xts(1)
            for lq in range(1, NLQ):
                nc.sync.dma_start(mod[:, lq], mod_r[lq])
            for q in range(2, NQ):
                load_xts(q)

            # PE p-state ramp burner while DMAs land (~107ns each; sized to
            # end right as wi+xts0's semaphores fire, keeping HAM at 8/8).
            for w in range(46):
                pw = pap.tile([128, 128], F32, name=f"warm{w}", tag="pa")
                nc.tensor.matmul(pw[:], warm[:], warm[:], start=True, stop=True)

            for q in range(NQ):
                lq = q % NLQ
                last = q == NQ - 1
                hms = []
                for j in range(NJ):
                    pa = pap.tile([128, QCH], F32, tag="pa")
                    for k in range(NK):
                        nc.tensor.matmul(
                            pa[:],
                            wi[:, k, 128 * j : 128 * (j + 1)],
                            xts_q[q][:, k, :],
                            start=(k == 0),
                            stop=(k == NK - 1),
                        )
                    hm = hmp.tile([128, QCH], BF16, tag="hm")
                    nc.vector.tensor_mul(hm[:], pa[:], mod[:, lq, j, :])
                    hms.append(hm)
                pys = [pyp.tile([128, QCH], F32, name=f"py{q}_{n}", tag="py")
                       for n in range(ND)]
                # bank-major accumulation on the last chunk so each PSUM
                # bank finishes early and its eviction overlaps the
                # remaining matmuls (shrinks the tail); elsewhere j-major
                # so the py phase starts as soon as hms[0] is ready.
                order = (
                    [(j, n) for n in range(ND) for j in range(NJ)]
                    if last else
                    [(j, n) for j in range(NJ) for n in range(ND)]
                )
                for j, n in order:
                    nc.tensor.matmul(
                        pys[n][:],
                        wo[:, j, 128 * n : 128 * (n + 1)],
                        hms[j][:],
                        start=(j == 0),
                        stop=(j == NJ - 1),
                    )
                # eviction: copies on ACT only (DVE stays muls-only so the
                # next chunk's multiplies never queue behind eviction), one
                # batched out-DMA per chunk. Last chunk: split copies across
                # ACT+DVE and the DMA across both HWDGE rings for a short
                # tail.
                yot = yop.tile([128, ND, QCH], BF16, tag="yo")
                for n in range(ND):
                    if last and n >= 2:
                        nc.vector.tensor_copy(yot[:, n, :], pys[n][:])
                    else:
                        nc.scalar.copy(yot[:, n, :], pys[n][:])
                if last:
                    nc.scalar.dma_start(yp_r[q][:, 0:2], yot[:, 0:2, :])
                    nc.sync.dma_start(yp_r[q][:, 2:4], yot[:, 2:4, :])
                else:
                    nc.scalar.dma_start(yp_r[q], yot[:])

    nc.finalize()
    return nc


def _get_nc():
    if "nc" not in _cache:
        _cache["nc"] = _build()
    return _cache["nc"]


def _bf(a):
    import ml_dtypes
    return np.ascontiguousarray(a.astype(ml_dtypes.bfloat16))


def _in_maps(x, input_proj, output_proj, floquet_energies, drive_weights,
             coupling_matrix):
    coupled = coupling_matrix.astype(np.float64) @ drive_weights.astype(np.float64)
    t = np.arange(L, dtype=np.float64) / L
    ang = 2.0 * np.pi * np.arange(1, M + 1, dtype=np.float64)[None, :] * t[:, None]
    C = (np.cos(ang) * coupled[None, :]).astype(np.float32)   # [L, M]
    S = (np.sin(ang) * coupled[None, :]).astype(np.float32)
    E = floquet_energies.astype(np.float64)
    mod = C @ np.cos(E).astype(np.float32) + S @ (-np.sin(E)).astype(np.float32)

    xT = _bf(x.reshape(TK, D).T)
    maps = []
    for c in range(NCORES):
        s = slice(c * HDS, (c + 1) * HDS)
        maps.append(
            {
                "xT": xT,
                "wi": _bf(input_proj[:, s]),
                "wo": _bf(output_proj[s, :]),
                "mod": _bf(mod[:, s].T),
            }
        )
    return maps


def kernel(x, input_proj, output_proj, floquet_energies, drive_weights,
           coupling_matrix, _trace=False, _trace_kwargs=None):
    from concourse.bass_utils import run_bass_kernel_spmd

    nc = _get_nc()
    maps = _in_maps(x, input_proj, output_proj, floquet_energies,
                    drive_weights, coupling_matrix)
    kw = dict(_trace_kwargs or {})
    res = run_bass_kernel_spmd(nc, maps, list(range(NCORES)), trace=_trace, **kw)
    acc = np.zeros((D, TK), dtype=np.float32)
    for c in range(NCORES):
        acc += res.results[c]["yp"].astype(np.float32)
    out = np.ascontiguousarray(acc.T).reshape(B, L, D)
    if _trace:
        return out, res
    return out
